# revision 14
# baseline (speedup 1.0000x reference)
"""FFT-based DCT-II on 8 trn2 NeuronCores (v3).

Per core (256 rows = 2 h-halves x 128): Makhoul DCT->real-FFT, four-step
radix-64x64. Stage 1 uses 64 slots (33 cos + 31 sin, zero columns dropped)
so the two h-halves col-tile into one 128-partition psum tile. Mid
transpose roundtrips DRAM with contiguous WRITE legs (4KB/partition) and
scattered READ legs (256B runs; HBM sub-512B penalty is write-side RMW,
reads only pay packet overhead). Stage 2 is 32 uniform matmuls (a=0/32
merged); y stores are contiguous fp16.

Layouts:
  x1[64h + n1, 128 n2 + r'] = v[128h + r', 64 n1 + n2]
  slots s = 32c + a (c-major): c=0 cos_a, c=1 -sin_a (cos_32 in sin_0 slot)
  t_sb  [128=(h,c,a), n2, r']         (psum partition order)
  t_dram[h, c, a, n2, r']             (write-contiguous)
  t2    [128=(c,n2), a, h, r']        (stage-2 rhs order)
  y2    [d, k2, a, (h r')]            (store-contiguous)
"""

import numpy as np

N = 4096
R = 2048
RPC = 256

_state = {}


def _tables():
    n1 = np.arange(64, dtype=np.float64)
    f1 = np.zeros((64, 64))
    a_ = np.arange(32, dtype=np.float64)
    f1[:, :32] = np.cos(2 * np.pi * n1[:, None] * a_[None, :] / 64)
    f1[:, 33:] = -np.sin(2 * np.pi * n1[:, None] * a_[None, 1:] / 64)
    f1[:, 32] = np.cos(np.pi * n1)  # cos_32 in the sin_0 slot
    f1_np = np.vstack([f1, f1]).astype(np.float16)  # [128, 64]

    n2 = np.arange(64, dtype=np.float64)[:, None]
    k2 = np.arange(64, dtype=np.float64)[None, :]
    hh2 = np.zeros((128, 32, 128))
    for a in range(32):
        for d in range(2):
            k1 = (a if d == 0 else 64 - a) if a >= 1 else (0 if d == 0 else 32)
            kk = 64 * k2 + k1
            th = np.pi * kk * (4 * n2 + 1) / 8192  # [n2, k2]
            cols = (64 * d + np.arange(64))[None, :]
            rows = np.arange(64)[:, None]
            if a == 0:
                # d=0 (k1=0) uses only c=0 rows (cos_0); d=1 (k1=32) uses
                # only c=1 rows (cos_32 parked in the sin_0 slot)
                hh2[64 * d + rows, 0, cols] = np.cos(th)
            else:
                sgn = 1.0 if d == 0 else -1.0
                hh2[rows, a, cols] = np.cos(th)
                hh2[64 + rows, a, cols] = sgn * np.sin(th)
    hh_np = hh2.astype(np.float16).copy()  # [128, 32, 128]

    k1_map = np.empty(64, dtype=np.int64)
    for a in range(32):
        for d in range(2):
            k1_map[2 * a + d] = (a if d == 0 else 64 - a) if a >= 1 else (
                0 if d == 0 else 32
            )
    return f1_np, hh_np, k1_map


def _build():
    import concourse.tile as tile
    from concourse import bacc, mybir

    f16 = mybir.dt.float16
    f32 = mybir.dt.float32

    nc = bacc.Bacc("TRN2", target_bir_lowering=False, debug=False, num_devices=8)
    x1_d = nc.dram_tensor("x1", [128, 8192], f16, kind="ExternalInput").ap()
    f1_d = nc.dram_tensor("f1", [128, 64], f16, kind="ExternalInput").ap()
    hh_d = nc.dram_tensor("hh", [128, 32, 128], f16, kind="ExternalInput").ap()
    y_d = nc.dram_tensor("y", [2, 64, 32, 256], f16, kind="ExternalOutput").ap()

    with tile.TileContext(nc) as tc:
        with (
            tc.tile_pool(name="const", bufs=1) as const,
            tc.tile_pool(name="data", bufs=1) as data,
            tc.tile_pool(name="dram", bufs=1, space="DRAM") as dram,
            tc.tile_pool(name="ps1", bufs=4, space="PSUM") as ps1,
            tc.tile_pool(name="ps2", bufs=2, space="PSUM") as ps2,
            tc.tile_pool(name="ysb", bufs=4) as ysbp,
        ):
            f1_sb = const.tile([128, 64], f16)
            hh_sb = const.tile([128, 32, 128], f16)
            # sync queue FIFO: f1, x chunks, then hh (so hh cannot steal
            # HBM bandwidth from x)
            nc.sync.dma_start(f1_sb[:], f1_d)
            x1_g = []
            for g in range(8):
                xg = data.tile([128, 1024], f16, name=f"x1_{g}")
                nc.sync.dma_start(xg[:], x1_d[:, 1024 * g : 1024 * g + 1024])
                x1_g.append(xg)
            nc.sync.dma_start(hh_sb[:], hh_d)

            t_sb = data.tile([128, 64, 128], f16)
            t_dram = dram.tile([2, 2, 32, 64, 128], f16)  # (h, c, a, n2, r')
            t2 = data.tile([128, 2, 32, 128], f16)  # (c n2), h, a, r'

            # stage 1: per 4-n2 half-chunk, one 1-bank psum tile (h0/h1
            # col-tiled pair), copy out; copies keep pace with matmuls.
            cb = 0
            for g in range(8):
                for u in range(2):
                    ps = ps1.tile([128, 512], f32)
                    for h in range(2):
                        nc.tensor.matmul(
                            ps[64 * h : 64 * h + 64, :],
                            f1_sb[64 * h : 64 * h + 64, :],
                            x1_g[g][64 * h : 64 * h + 64, 512 * u : 512 * u + 512],
                            start=True,
                            stop=True,
                        )
                    n0 = 8 * g + 4 * u
                    dst = t_sb[:, n0 : n0 + 4, :]
                    src = ps[:].rearrange("p (n r) -> p n r", n=4)
                    if cb % 2 == 0:
                        nc.vector.tensor_copy(dst, src)
                    else:
                        nc.scalar.copy(dst, src)
                    cb += 1
                # transpose write legs (contiguous 4KB+ dst runs), two
                # 32-n2 waves on the HWDGE queues (c0->sync, c1->scalar)
                if g % 4 == 3:
                    n0 = 32 * (g // 4)
                    for c in range(2):
                        for h in range(2):
                            src = t_sb[
                                64 * h + 32 * c : 64 * h + 32 * c + 32,
                                n0 : n0 + 32,
                                :,
                            ]
                            dst = t_dram[h, c, :, n0 : n0 + 32, :]
                            if c == 0:
                                nc.sync.dma_start(dst, src)
                            else:
                                nc.scalar.dma_start(dst, src)

            # transpose read legs: per (c, h, n2-half, a-halfchunk); 256B
            # DRAM-read runs (no RMW) into 2KB-contiguous SBUF lines.
            # n2-half 0 depends only on write wave 1, overlapping stage 1.
            for nh in range(2):
                for j2 in range(2):
                    for c in range(2):
                        for h in range(2):
                            src = t_dram[
                                h, c, 16 * j2 : 16 * j2 + 16, 32 * nh : 32 * nh + 32, :
                            ].rearrange("a n r -> n a r")
                            dst = t2[
                                64 * c + 32 * nh : 64 * c + 32 * nh + 32,
                                h,
                                16 * j2 : 16 * j2 + 16,
                                :,
                            ]
                            if c == 0:
                                nc.sync.dma_start(dst, src)
                            else:
                                nc.scalar.dma_start(dst, src)

            # stage 2: per q, 4 groups into one 2-bank psum; copy; store.
            for q in range(8):
                ps = ps2.tile([128, 1024], f32)
                for i in range(4):
                    a = 4 * q + i
                    nc.tensor.matmul(
                        ps[:, 256 * i : 256 * i + 256],
                        hh_sb[:, a, :],
                        t2[:, :, a, :],
                        start=True,
                        stop=True,
                    )
                y_sb = ysbp.tile([128, 4, 256], f16)
                src = ps[:].rearrange("p (a r) -> p a r", a=4)
                if q % 2 == 0:
                    nc.vector.tensor_copy(y_sb[:], src)
                else:
                    nc.scalar.copy(y_sb[:], src)
                dst = y_d[:, :, 4 * q : 4 * q + 4, :].rearrange(
                    "d k a r -> (d k) a r"
                )
                nc.gpsimd.dma_start(dst, y_sb[:])

    nc.compile()
    return nc


def _pack_x1(x_rows):
    v = np.empty_like(x_rows)
    v[:, : N // 2] = x_rows[:, 0::2]
    v[:, N // 2 :] = x_rows[:, 1::2][:, ::-1]
    x1 = v.reshape(2, 128, 64, 64).transpose(0, 2, 3, 1).reshape(128, 8192)
    return np.ascontiguousarray(x1.astype(np.float16))


def kernel(x, _trace: bool = False):
    from concourse.bass_utils import run_bass_kernel_spmd

    x = np.asarray(x, dtype=np.float32)
    assert x.shape == (R, N)
    if "nc" not in _state:
        _state["nc"] = _build()
        _state["tables"] = _tables()
    nc = _state["nc"]
    f1_np, hh_np, k1_map = _state["tables"]

    in_maps = []
    for c in range(8):
        in_maps.append(
            {
                "x1": _pack_x1(x[c * RPC : (c + 1) * RPC]),
                "f1": f1_np,
                "hh": hh_np,
            }
        )

    res = run_bass_kernel_spmd(nc, in_maps, list(range(8)), trace=_trace)

    y = np.empty((R, N), dtype=np.float32)
    for c in range(8):
        ydev = res.results[c]["y"]  # [2, 64, 32, 256] fp16 = (d, k2, a, (h r'))
        # y[128h + r', 64 k2 + k1(a, d)] = ydev[d, k2, a, 128h + r']
        perm = np.asarray(ydev, dtype=np.float32).transpose(3, 1, 2, 0)
        perm = perm.reshape(RPC, 64, 64)  # (r_full, k2, (a d))
        yc = np.empty((RPC, 64, 64), dtype=np.float32)
        yc[:, :, k1_map] = perm
        y[c * RPC : (c + 1) * RPC] = yc.reshape(RPC, N)
    if _trace:
        _state["last_result"] = res
    return y


# revision 18
# speedup vs baseline: 1.0127x; 1.0127x over previous
"""FFT-based DCT-II on 8 trn2 NeuronCores (v3).

Per core (256 rows = 2 h-halves x 128): Makhoul DCT->real-FFT, four-step
radix-64x64. Stage 1 uses 64 slots (33 cos + 31 sin, zero columns dropped)
so the two h-halves col-tile into one 128-partition psum tile. Mid
transpose roundtrips DRAM with contiguous WRITE legs (4KB/partition) and
scattered READ legs (256B runs; HBM sub-512B penalty is write-side RMW,
reads only pay packet overhead). Stage 2 is 32 uniform matmuls (a=0/32
merged); y stores are contiguous fp16.

Layouts:
  x1[64h + n1, 128 n2 + r'] = v[128h + r', 64 n1 + n2]
  slots s = 32c + a (c-major): c=0 cos_a, c=1 -sin_a (cos_32 in sin_0 slot)
  t_sb  [128=(h,c,a), n2, r']         (psum partition order)
  t_dram[h, c, a, n2, r']             (write-contiguous)
  t2    [128=(c,n2), a, h, r']        (stage-2 rhs order)
  y2    [d, k2, a, (h r')]            (store-contiguous)
"""

import numpy as np

N = 4096
R = 2048
RPC = 256

_state = {}


def _tables():
    n1 = np.arange(64, dtype=np.float64)
    f1 = np.zeros((64, 64))
    a_ = np.arange(32, dtype=np.float64)
    f1[:, :32] = np.cos(2 * np.pi * n1[:, None] * a_[None, :] / 64)
    f1[:, 33:] = -np.sin(2 * np.pi * n1[:, None] * a_[None, 1:] / 64)
    f1[:, 32] = np.cos(np.pi * n1)  # cos_32 in the sin_0 slot
    f1_np = np.vstack([f1, f1]).astype(np.float16)  # [128, 64]

    n2 = np.arange(64, dtype=np.float64)[:, None]
    k2 = np.arange(64, dtype=np.float64)[None, :]
    hh2 = np.zeros((128, 32, 128))
    for a in range(32):
        for d in range(2):
            k1 = (a if d == 0 else 64 - a) if a >= 1 else (0 if d == 0 else 32)
            kk = 64 * k2 + k1
            th = np.pi * kk * (4 * n2 + 1) / 8192  # [n2, k2]
            cols = (64 * d + np.arange(64))[None, :]
            rows = np.arange(64)[:, None]
            if a == 0:
                # d=0 (k1=0) uses only c=0 rows (cos_0); d=1 (k1=32) uses
                # only c=1 rows (cos_32 parked in the sin_0 slot)
                hh2[64 * d + rows, 0, cols] = np.cos(th)
            else:
                sgn = 1.0 if d == 0 else -1.0
                hh2[rows, a, cols] = np.cos(th)
                hh2[64 + rows, a, cols] = sgn * np.sin(th)
    hh_np = hh2.astype(np.float16).copy()  # [128, 32, 128]

    k1_map = np.empty(64, dtype=np.int64)
    for a in range(32):
        for d in range(2):
            k1_map[2 * a + d] = (a if d == 0 else 64 - a) if a >= 1 else (
                0 if d == 0 else 32
            )
    return f1_np, hh_np, k1_map


def _build():
    import concourse.tile as tile
    from concourse import bacc, mybir

    f16 = mybir.dt.float16
    f32 = mybir.dt.float32

    nc = bacc.Bacc("TRN2", target_bir_lowering=False, debug=False, num_devices=8)
    x1_d = nc.dram_tensor("x1", [128, 8192], f16, kind="ExternalInput").ap()
    f1_d = nc.dram_tensor("f1", [128, 64], f16, kind="ExternalInput").ap()
    hh_d = nc.dram_tensor("hh", [128, 32, 128], f16, kind="ExternalInput").ap()
    y_d = nc.dram_tensor("y", [2, 64, 32, 256], f16, kind="ExternalOutput").ap()

    with tile.TileContext(nc) as tc:
        with (
            tc.tile_pool(name="const", bufs=1) as const,
            tc.tile_pool(name="data", bufs=1) as data,
            tc.tile_pool(name="dram", bufs=1, space="DRAM") as dram,
            tc.tile_pool(name="ps1", bufs=4, space="PSUM") as ps1,
            tc.tile_pool(name="ps2", bufs=4, space="PSUM") as ps2,
            tc.tile_pool(name="ysb", bufs=4) as ysbp,
        ):
            f1_sb = const.tile([128, 64], f16)
            hh_sb = const.tile([128, 32, 128], f16)
            # sync queue FIFO: f1, x chunks, then hh (so hh cannot steal
            # HBM bandwidth from x)
            nc.sync.dma_start(f1_sb[:], f1_d)
            x1_g = []
            for g in range(8):
                xg = data.tile([128, 1024], f16, name=f"x1_{g}")
                nc.sync.dma_start(xg[:], x1_d[:, 1024 * g : 1024 * g + 1024])
                x1_g.append(xg)
            # hh on the gpsimd queue: keeps it out of the sync queue's
            # FIFO so transpose write wave 1 isn't stuck behind its 1MB
            nc.gpsimd.dma_start(hh_sb[:], hh_d)

            t_sb = data.tile([128, 64, 128], f16)
            t_dram = dram.tile([2, 2, 32, 64, 128], f16)  # (h, c, a, n2, r')
            t2 = data.tile([128, 2, 32, 128], f16)  # (c n2), h, a, r'

            # stage 1: per 4-n2 half-chunk, one 1-bank psum tile (h0/h1
            # col-tiled pair), copy out; copies keep pace with matmuls.
            cb = 0
            for g in range(8):
                for u in range(2):
                    ps = ps1.tile([128, 512], f32)
                    for h in range(2):
                        nc.tensor.matmul(
                            ps[64 * h : 64 * h + 64, :],
                            f1_sb[64 * h : 64 * h + 64, :],
                            x1_g[g][64 * h : 64 * h + 64, 512 * u : 512 * u + 512],
                            start=True,
                            stop=True,
                        )
                    n0 = 8 * g + 4 * u
                    dst = t_sb[:, n0 : n0 + 4, :]
                    src = ps[:].rearrange("p (n r) -> p n r", n=4)
                    if cb % 2 == 0:
                        nc.vector.tensor_copy(dst, src)
                    else:
                        nc.scalar.copy(dst, src)
                    cb += 1
                # transpose write legs (contiguous 4KB+ dst runs), two
                # 32-n2 waves on the HWDGE queues (c0->sync, c1->scalar)
                if g % 4 == 3:
                    n0 = 32 * (g // 4)
                    for c in range(2):
                        for h in range(2):
                            src = t_sb[
                                64 * h + 32 * c : 64 * h + 32 * c + 32,
                                n0 : n0 + 32,
                                :,
                            ]
                            dst = t_dram[h, c, :, n0 : n0 + 32, :]
                            if c == 0:
                                nc.sync.dma_start(dst, src)
                            else:
                                nc.scalar.dma_start(dst, src)

            # transpose read legs: per (c, h, n2-half, a-halfchunk); 256B
            # DRAM-read runs (no RMW) into 2KB-contiguous SBUF lines.
            # n2-half 0 depends only on write wave 1, overlapping stage 1.
            for j2 in range(2):
                for nh in range(2):
                    for c in range(2):
                        for h in range(2):
                            src = t_dram[
                                h, c, 16 * j2 : 16 * j2 + 16, 32 * nh : 32 * nh + 32, :
                            ].rearrange("a n r -> n a r")
                            dst = t2[
                                64 * c + 32 * nh : 64 * c + 32 * nh + 32,
                                h,
                                16 * j2 : 16 * j2 + 16,
                                :,
                            ]
                            if c == 0:
                                nc.sync.dma_start(dst, src)
                            else:
                                nc.scalar.dma_start(dst, src)

            # stage 2: per q, 2 groups into one 1-bank psum; copy; store.
            for q in range(16):
                ps = ps2.tile([128, 512], f32)
                for i in range(2):
                    a = 2 * q + i
                    nc.tensor.matmul(
                        ps[:, 256 * i : 256 * i + 256],
                        hh_sb[:, a, :],
                        t2[:, :, a, :],
                        start=True,
                        stop=True,
                    )
                y_sb = ysbp.tile([128, 2, 256], f16)
                src = ps[:].rearrange("p (a r) -> p a r", a=2)
                if q % 2 == 0:
                    nc.vector.tensor_copy(y_sb[:], src)
                else:
                    nc.scalar.copy(y_sb[:], src)
                dst = y_d[:, :, 2 * q : 2 * q + 2, :].rearrange(
                    "d k a r -> (d k) a r"
                )
                nc.gpsimd.dma_start(dst, y_sb[:])

    nc.compile()
    return nc


def _pack_x1(x_rows):
    v = np.empty_like(x_rows)
    v[:, : N // 2] = x_rows[:, 0::2]
    v[:, N // 2 :] = x_rows[:, 1::2][:, ::-1]
    x1 = v.reshape(2, 128, 64, 64).transpose(0, 2, 3, 1).reshape(128, 8192)
    return np.ascontiguousarray(x1.astype(np.float16))


def kernel(x, _trace: bool = False):
    from concourse.bass_utils import run_bass_kernel_spmd

    x = np.asarray(x, dtype=np.float32)
    assert x.shape == (R, N)
    if "nc" not in _state:
        _state["nc"] = _build()
        _state["tables"] = _tables()
    nc = _state["nc"]
    f1_np, hh_np, k1_map = _state["tables"]

    in_maps = []
    for c in range(8):
        in_maps.append(
            {
                "x1": _pack_x1(x[c * RPC : (c + 1) * RPC]),
                "f1": f1_np,
                "hh": hh_np,
            }
        )

    res = run_bass_kernel_spmd(nc, in_maps, list(range(8)), trace=_trace)

    y = np.empty((R, N), dtype=np.float32)
    for c in range(8):
        ydev = res.results[c]["y"]  # [2, 64, 32, 256] fp16 = (d, k2, a, (h r'))
        # y[128h + r', 64 k2 + k1(a, d)] = ydev[d, k2, a, 128h + r']
        perm = np.asarray(ydev, dtype=np.float32).transpose(3, 1, 2, 0)
        perm = perm.reshape(RPC, 64, 64)  # (r_full, k2, (a d))
        yc = np.empty((RPC, 64, 64), dtype=np.float32)
        yc[:, :, k1_map] = perm
        y[c * RPC : (c + 1) * RPC] = yc.reshape(RPC, N)
    if _trace:
        _state["last_result"] = res
    return y
